# revision 2
# baseline (speedup 1.0000x reference)
"""CME cell Trainium kernel (nn_CMECell): til[t,d,tau] = Re(eta . F[d,tau,:]) / tau
with complex state F[d,tau,k] following F = F*exp(-s*a_t) + f_t*exprel(-s*a_t).

Reformulation (rotating frame): with s[tau,k] = rho_tau*(1 + i*k*omega),
exact phases psi_t = omega*rho*A_t (A = cumsum alpha, host f64, range-reduced),
the complex recurrence decouples into REAL scans with shared decay
dcy_t = exp(-rho*a_t):
    Sre/Sim[k]_t = dcy_t*S_{t-1} + dw_t*(p_k/q_k)(t),   dw_t = w_t - w_{t+1},
    w_t = f_t/(rho*a_t),  (p+iq)_k = E2_k*e^{i k psi},  E2_k = eta_k/(tau*(1+ik*omega))
    til_t = sum_k [cos(k psi_t)*Sre_k + sin(k psi_t)*Sim_k] + w_{t+1}*sum_k Re(E2_k)

Sharding: D=64 channels over 8 cores (8 each); per core partition-tiles of
(2 channels x 50 taus) = 100 partitions, time in blocks of 512 on the free axis.
Trig via fp32 magic-round frac(k*phi) + ACT Sin (domain +-pi).
"""
import sys
sys.path.insert(0, "/opt/trn_rl_repo")
import numpy as np

import concourse.bass as bass
from concourse import bacc
import concourse.mybir as mybir
from concourse import tile
from concourse.bass_utils import run_bass_kernel_spmd

T, D, NT, NK = 4096, 64, 50, 10
NCORES = 8
DL = D // NCORES          # channels per core = 8
P = 100                   # partitions per tile: 2 channels x 50 taus
NPT = DL // 2             # partition tiles per core = 4
L = 512                   # time block
NB = T // L
AMIN = np.float32(3e-5)
MAGIC = float(2.0 ** 23)
TWO_PI = float(2 * np.pi)
DT = mybir.dt.float32
AF = mybir.ActivationFunctionType
OP = mybir.AluOpType

_built = {}


def _build_nc():
    if "nc" in _built:
        return _built["nc"]
    nc = bacc.Bacc(None, target_bir_lowering=False)

    phi_in = nc.declare_dram_parameter("phi", [NPT, P, T + 1], DT, isOutput=False)
    a_in = nc.declare_dram_parameter("aT", [DL, T], DT, isOutput=False)
    w_in = nc.declare_dram_parameter("wbe", [DL, T + 2], DT, isOutput=False)
    sel_in = nc.declare_dram_parameter("sel", [DL, 2 * NPT * P], DT, isOutput=False)
    e2_in = nc.declare_dram_parameter("e2", [P, 21], DT, isOutput=False)
    out_dr = nc.declare_dram_parameter("out", [NPT * P, T], DT, isOutput=True)

    with tile.TileContext(nc) as tc:
        with (
            tc.tile_pool(name="consts", bufs=1) as consts,
            tc.tile_pool(name="carryp", bufs=1) as carryp,
            tc.tile_pool(name="io", bufs=3) as io,
            tc.tile_pool(name="trig", bufs=2) as trigp,
            tc.tile_pool(name="work", bufs=3) as work,
            tc.tile_pool(name="scanp", bufs=3) as scanp,
            tc.tile_pool(name="psx", bufs=2, space="PSUM") as psx,
            tc.tile_pool(name="psw", bufs=2, space="PSUM") as psw,
        ):
            selt = consts.tile([DL, 2 * NPT * P], DT)
            e2t = consts.tile([P, 21], DT)
            at_t = consts.tile([DL, T], DT)
            wbe_t = consts.tile([DL, T + 2], DT)
            nc.sync.dma_start(selt[:], sel_in[:])
            nc.sync.dma_start(e2t[:], e2_in[:])
            nc.sync.dma_start(at_t[:], a_in[:])
            nc.sync.dma_start(wbe_t[:], w_in[:])

            carries = []
            for pt in range(NPT):
                ct = carryp.tile([P, 20], DT, tag=f"carry{pt}")
                carries.append(ct)

            def sel_x(pt):
                return selt[:, (0 * NPT + pt) * P:(0 * NPT + pt + 1) * P]

            def sel_w(pt):
                return selt[:, (1 * NPT + pt) * P:(1 * NPT + pt + 1) * P]

            for b in range(NB):
                first = b == 0
                last_b = b == NB - 1
                t0 = b * L
                E = L + 1 if first else L          # scan length
                WC = L + 2 if first else L + 1     # w columns
                wlo = t0 if first else t0 + 1      # first wbe col
                for pt in range(NPT):
                    carry = carries[pt]
                    # ---- PE: x and w broadcasts
                    x_ps = psx.tile([P, L], DT, tag="x")
                    nc.tensor.matmul(x_ps[:], sel_x(pt), at_t[:, t0:t0 + L],
                                     start=True, stop=True)
                    w_ps = psw.tile([P, L + 2], DT, tag="w")
                    nc.tensor.matmul(w_ps[:, 0:512], sel_w(pt),
                                     wbe_t[:, wlo:wlo + 512], start=True, stop=True)
                    nc.tensor.matmul(w_ps[:, 512:WC], sel_w(pt),
                                     wbe_t[:, wlo + 512:wlo + WC],
                                     start=True, stop=True)
                    w_sb = work.tile([P, L + 2], DT, tag="wsb")
                    nc.vector.tensor_copy(w_sb[:, 0:WC], w_ps[:, 0:WC])

                    # ---- decay
                    dcy = work.tile([P, E], DT, tag="dcy")
                    if first:
                        nc.vector.memset(dcy[:, 0:1], 1.0)
                        nc.scalar.activation(dcy[:, 1:E], x_ps[:], AF.Exp, scale=-1.0)
                    else:
                        nc.scalar.activation(dcy[:], x_ps[:], AF.Exp, scale=-1.0)

                    # ---- dw
                    dw = work.tile([P, E], DT, tag="dw")
                    nc.vector.tensor_tensor(dw[:], w_sb[:, 0:E], w_sb[:, 1:E + 1],
                                            op=OP.subtract)

                    # ---- phi block
                    phit = io.tile([P, L + 1], DT, tag="phi")
                    nc.sync.dma_start(phit[:], phi_in[pt, :, t0:t0 + L + 1])
                    c0 = 0 if first else 1   # trig col offset so width == E

                    # ---- k = 0 scan -> til base
                    u0 = work.tile([P, E], DT, tag="u0")
                    nc.vector.tensor_scalar(u0[:], dw[:], e2t[:, 0:1], None,
                                            op0=OP.mult)
                    s0 = scanp.tile([P, E], DT, tag="s0")
                    init0 = 0.0 if first else carry[:, 0:1]
                    nc.vector.tensor_tensor_scan(s0[:], dcy[:], u0[:], init0,
                                                 op0=OP.mult, op1=OP.add)
                    if not last_b:
                        nc.vector.tensor_copy(carry[:, 0:1], s0[:, E - 1:E])

                    sl = slice(1, L + 1) if first else slice(0, L)
                    til = work.tile([P, L], DT, tag="til")
                    nc.vector.tensor_copy(til[:], s0[:, sl])

                    # ---- k = 1..9
                    for k in range(1, NK):
                        kf = float(k)
                        ys = trigp.tile([P, L + 1], DT, tag="ys")
                        nc.vector.tensor_scalar(ys[:], phit[:], kf, None,
                                                op0=OP.mult)
                        rs = trigp.tile([P, L + 1], DT, tag="rs")
                        nc.vector.tensor_scalar(rs[:], ys[:], MAGIC, MAGIC,
                                                op0=OP.add, op1=OP.subtract)
                        us = trigp.tile([P, L + 1], DT, tag="us")
                        nc.vector.scalar_tensor_tensor(us[:], rs[:], -1.0, ys[:],
                                                       op0=OP.mult, op1=OP.add)
                        yc = trigp.tile([P, L + 1], DT, tag="yc")
                        nc.vector.tensor_scalar(yc[:], phit[:], kf, 0.25,
                                                op0=OP.mult, op1=OP.add)
                        rc = trigp.tile([P, L + 1], DT, tag="rc")
                        nc.vector.tensor_scalar(rc[:], yc[:], MAGIC, MAGIC,
                                                op0=OP.add, op1=OP.subtract)
                        uc = trigp.tile([P, L + 1], DT, tag="uc")
                        nc.vector.scalar_tensor_tensor(uc[:], rc[:], -1.0, yc[:],
                                                       op0=OP.mult, op1=OP.add)
                        S_k = trigp.tile([P, L + 1], DT, tag="S")
                        C_k = trigp.tile([P, L + 1], DT, tag="C")
                        nc.scalar.activation(S_k[:], us[:], AF.Sin, scale=TWO_PI)
                        nc.scalar.activation(C_k[:], uc[:], AF.Sin, scale=TWO_PI)

                        # p,q = E2-rotated trig on cols [c0, L+1) (width E)
                        CV = C_k[:, c0:L + 1]
                        SV = S_k[:, c0:L + 1]
                        tm1 = work.tile([P, E], DT, tag="tm1")
                        nc.vector.tensor_scalar(tm1[:], SV, e2t[:, 10 + k:11 + k],
                                                None, op0=OP.mult)
                        p_k = work.tile([P, E], DT, tag="pk")
                        nc.vector.scalar_tensor_tensor(p_k[:], CV,
                                                       e2t[:, k:k + 1], tm1[:],
                                                       op0=OP.mult,
                                                       op1=OP.subtract)
                        tm2 = work.tile([P, E], DT, tag="tm2")
                        nc.vector.tensor_scalar(tm2[:], CV, e2t[:, 10 + k:11 + k],
                                                None, op0=OP.mult)
                        q_k = work.tile([P, E], DT, tag="qk")
                        nc.vector.scalar_tensor_tensor(q_k[:], SV,
                                                       e2t[:, k:k + 1], tm2[:],
                                                       op0=OP.mult, op1=OP.add)

                        ure = work.tile([P, E], DT, tag="ure")
                        nc.vector.tensor_tensor(ure[:], dw[:], p_k[:], op=OP.mult)
                        uim = work.tile([P, E], DT, tag="uim")
                        nc.vector.tensor_tensor(uim[:], dw[:], q_k[:], op=OP.mult)

                        sre = scanp.tile([P, E], DT, tag="sre")
                        sim = scanp.tile([P, E], DT, tag="sim")
                        init_re = 0.0 if first else carry[:, k:k + 1]
                        init_im = 0.0 if first else carry[:, 10 + k:11 + k]
                        nc.vector.tensor_tensor_scan(sre[:], dcy[:], ure[:],
                                                     init_re, op0=OP.mult,
                                                     op1=OP.add)
                        nc.vector.tensor_tensor_scan(sim[:], dcy[:], uim[:],
                                                     init_im, op0=OP.mult,
                                                     op1=OP.add)
                        if not last_b:
                            nc.vector.tensor_copy(carry[:, k:k + 1],
                                                  sre[:, E - 1:E])
                            nc.vector.tensor_copy(carry[:, 10 + k:11 + k],
                                                  sim[:, E - 1:E])

                        # til += C*sre + S*sim  (on gpsimd to offload DVE)
                        tcol = c0 + sl.start   # == 1 always: phases t0+1..t0+L
                        tmc = work.tile([P, L], DT, tag="tmc")
                        nc.gpsimd.tensor_tensor(tmc[:], C_k[:, tcol:tcol + L],
                                                sre[:, sl], op=OP.mult)
                        nc.gpsimd.tensor_tensor(til[:], til[:], tmc[:], op=OP.add)
                        tms = work.tile([P, L], DT, tag="tms")
                        nc.gpsimd.tensor_tensor(tms[:], S_k[:, tcol:tcol + L],
                                                sim[:, sl], op=OP.mult)
                        nc.gpsimd.tensor_tensor(til[:], til[:], tms[:], op=OP.add)

                    # ---- correction: til += w_{t+1} * sum_k E2re
                    wplus = w_sb[:, (2 if first else 1):(2 if first else 1) + L]
                    tilf = work.tile([P, L], DT, tag="tilf")
                    nc.vector.scalar_tensor_tensor(tilf[:], wplus,
                                                   e2t[:, 20:21], til[:],
                                                   op0=OP.mult, op1=OP.add)
                    nc.sync.dma_start(out_dr[pt * P:(pt + 1) * P, t0:t0 + L],
                                      tilf[:])

    nc.finalize()
    _built["nc"] = nc
    return nc


def _host_prep(f, alpha, s, eta, tau_stars):
    f = np.asarray(f, np.float32)
    alpha = np.asarray(alpha, np.float32)
    s = np.asarray(s)
    eta64 = np.asarray(eta).astype(np.complex128)
    tau64 = np.asarray(tau_stars).astype(np.float64)

    sr = s.real.astype(np.float64)
    si = s.imag.astype(np.float64)
    rho = sr[:, 0].copy()
    if not np.allclose(sr, rho[:, None], rtol=1e-5, atol=1e-30):
        raise ValueError("Re(s) varies with k; kernel assumes s = rho*(1+ik*omega)")
    with np.errstate(divide="ignore", invalid="ignore"):
        om = si[:, 1:] / (sr[:, 1:] * np.arange(1, NK)[None, :])
    omega = float(np.mean(om))
    kk = np.arange(NK)
    if not np.allclose(si, np.outer(rho, kk * omega), rtol=1e-4, atol=1e-25):
        raise ValueError("Im(s) not proportional to k*rho*omega")

    E2 = eta64[None, :] / (tau64[:, None] * (1.0 + 1j * kk[None, :] * omega))
    E2re = E2.real.astype(np.float32)          # (NT, NK)
    E2im = E2.imag.astype(np.float32)
    E2sum = E2.real.sum(axis=1).astype(np.float32)   # (NT,)

    a = np.maximum(alpha, AMIN)                # (T, D)
    w = (f / a).astype(np.float32)             # w at array col j <-> step j+1
    A64 = np.cumsum(a.astype(np.float64), axis=0)
    Aext = np.concatenate([np.zeros((1, D)), A64], axis=0)   # (T+1, D)

    # phi[t, d, tau] would be huge; build per-core slices directly
    rho32 = rho.astype(np.float32)

    in_maps = []
    for c in range(NCORES):
        dsl = slice(c * DL, (c + 1) * DL)
        aT = np.ascontiguousarray(a[:, dsl].T)                      # (DL, T)
        wbe = np.zeros((DL, T + 2), np.float32)
        wbe[:, 1:T + 1] = w[:, dsl].T
        # phi: (NPT, P, T+1) rows (dd, tau)
        Ac = Aext[:, dsl]                                           # (T+1, DL)
        phi = np.empty((NPT, P, T + 1), np.float32)
        for pt in range(NPT):
            for dd in range(2):
                dglob = 2 * pt + dd
                # (T+1, NT): omega*rho*A mod 2pi -> /2pi in [0,1)
                ph = np.mod(omega * rho[None, :] * Ac[:, dglob][:, None],
                            2 * np.pi) / (2 * np.pi)
                phi[pt, dd * NT:(dd + 1) * NT, :] = ph.T.astype(np.float32)
        sel = np.zeros((DL, 2 * NPT * P), np.float32)
        for pt in range(NPT):
            for dd in range(2):
                rows = slice(dd * NT, (dd + 1) * NT)
                sel[2 * pt + dd, (0 * NPT + pt) * P + dd * NT:
                    (0 * NPT + pt) * P + (dd + 1) * NT] = rho32
                sel[2 * pt + dd, (1 * NPT + pt) * P + dd * NT:
                    (1 * NPT + pt) * P + (dd + 1) * NT] = (1.0 / rho32)
        e2 = np.zeros((P, 21), np.float32)
        for dd in range(2):
            e2[dd * NT:(dd + 1) * NT, 0:NK] = E2re
            e2[dd * NT:(dd + 1) * NT, NK:2 * NK] = E2im
            e2[dd * NT:(dd + 1) * NT, 20] = E2sum
        in_maps.append({"phi": phi, "aT": aT, "wbe": wbe, "sel": sel, "e2": e2})
    return in_maps


def kernel(f, alpha, F0, s, eta, tau_stars):
    assert np.allclose(np.asarray(F0), 0), "kernel assumes F0 == 0"
    nc = _build_nc()
    in_maps = _host_prep(f, alpha, s, eta, tau_stars)
    res = run_bass_kernel_spmd(nc, in_maps, list(range(NCORES)))
    out = np.empty((T, D, NT), np.float32)
    for c in range(NCORES):
        o = res.results[c]["out"]                  # (NPT*P, T)
        o = o.reshape(NPT, 2, NT, T)               # (pt, dd, tau, t)
        # d_local = 2*pt + dd
        o = o.transpose(3, 0, 1, 2).reshape(T, DL, NT)
        out[:, c * DL:(c + 1) * DL, :] = o
    return out
